# revision 11
# baseline (speedup 1.0000x reference)
"""GCL (GNN message-passing) Trainium2 Bass kernel on 8 NeuronCores.

Sharding: edges sorted by destination on host and sharded by destination-node
range (1250 nodes/core) -> each core owns the full segment-sum for its nodes,
no collectives. Node features and weights replicated.

Per core, per 128-destination-node window, edges are processed in 512-edge
macro tiles:
  e1T[D,e] = A_hi/A_lo @ S_T + I @ Bcol          (PSUM accumulate)
  where A = h@we1_top + be1 (double-bf16, resident SBUF),
        B = h@we1_bot (single-bf16 HBM table, per-edge dma_gather on col),
        S_T[n,e] = one-hot(row_local[e]==n) built via K=1 broadcast matmul
                   + DVE is_equal against a partition-iota.
  e2[e,D] = silu(e1) @ we2 + be2                 (be2 via K=1 ones x be2)
  aggT[D,n] += e2^T-scatter via lhsT=e2, rhs=S   (PSUM accumulate per window)
Node MLP + residual per 128-node tile, fp32.
"""
import sys
sys.path.insert(0, '/opt/trn_rl_repo')
import numpy as np
import ml_dtypes

N_NODES = 10000
N_EDGES = 640000
D = 128
NORM = 100.0
NCORES = 8
NPC = N_NODES // NCORES          # 1250 destination nodes per core
NWIN = 10                        # 128-node windows per core
CALL = 1024                      # edges per dma_gather call (= 2 macros)
MACRO = 512
PAD_ROWLOCAL = 200.0

BF16 = ml_dtypes.bfloat16
_prog_cache = {}


def _wrap_idx16(idx):
    """[n] int -> [128, n/16] int16 wrapped (pos i -> partition i%16, col
    i//16) and replicated into all eight 16-partition groups."""
    n = idx.shape[0]
    block = idx.astype(np.int16).reshape(n // 16, 16).T
    return np.tile(block, (8, 1))


def _build_program(cw_per_window):
    import concourse.bacc as bacc
    import concourse.mybir as mybir
    from concourse import tile

    dt = mybir.dt
    AF = mybir.ActivationFunctionType
    ALU = mybir.AluOpType

    nm_w = [2 * c for c in cw_per_window]
    NM = sum(nm_w)
    NCALLS = sum(cw_per_window)
    RBLK = (NM + 127) // 128

    nc = bacc.Bacc("TRN2", target_bir_lowering=False, debug=False,
                   num_devices=NCORES)

    f32, bf16, i16 = dt.float32, dt.bfloat16, dt.int16
    din = lambda n, s, d=f32: nc.dram_tensor(n, s, d, kind="ExternalInput")
    hT = din("hT", [128, 10240])
    hT_slice = din("hT_slice", [128, NWIN * 128])
    h_slice = din("h_slice", [NWIN, 128, 128])
    we1_top = din("we1_top", [128, 128])
    we1_bot = din("we1_bot", [128, 128])
    be1_row = din("be1_row", [1, 128])
    we2_d = din("we2", [128, 128])
    be2rep4 = din("be2rep4", [1, 512])
    wn1_lo = din("wn1_lo", [128, 128])
    wn1_hi = din("wn1_hi", [128, 128])
    bn1_col = din("bn1_col", [128, 1])
    wn2_d = din("wn2", [128, 128])
    bn2_row = din("bn2_row", [1, 128])
    ones_row = din("ones_row", [1, 128])
    iota_col_d = din("iota_col", [128, 512])
    iota_part_d = din("iota_part", [128, 1])
    ident_bf_d = din("ident_bf", [128, 128], bf16)
    colidx_d = din("colidx", [128, 64 * NCALLS], i16)
    rowloc_c_d = din("rowloc_c", [128, 4 * NM])
    rowloc_r_d = din("rowloc_r", [NM, 512])
    out_d = nc.dram_tensor("out", [NWIN, 128, 128], f32, kind="ExternalOutput")

    NB = 80                                   # B table: 80*128 = 10240 rows
    B_hbm = nc.dram_tensor("B_scratch", [NB * 128, 128], bf16)

    with tile.TileContext(nc) as tc:
        with (
            tc.tile_pool(name="persist", bufs=1) as pp,
            tc.tile_pool(name="work", bufs=3) as wp,
            tc.tile_pool(name="gout", bufs=4) as gp,
            tc.tile_pool(name="ps", bufs=2, space="PSUM") as psp,
        ):
            def load(t_dram, shape, dtype=f32):
                t = pp.tile(shape, dtype, tag=t_dram.name)
                nc.sync.dma_start(t[:], t_dram.ap())
                return t

            hT_t = load(hT, [128, 10240])
            hTs_t = load(hT_slice, [128, NWIN * 128])
            colidx_t = load(colidx_d, [128, 64 * NCALLS], i16)
            rowloc_c = load(rowloc_c_d, [128, 4 * NM])
            w1t = load(we1_top, [128, 128])
            w1b = load(we1_bot, [128, 128])
            be1r = load(be1_row, [1, 128])
            w2 = load(we2_d, [128, 128])
            be2r = load(be2rep4, [1, 512])
            wn1l = load(wn1_lo, [128, 128])
            wn1h = load(wn1_hi, [128, 128])
            bn1c = load(bn1_col, [128, 1])
            wn2t = load(wn2_d, [128, 128])
            bn2r = load(bn2_row, [1, 128])
            onesr = load(ones_row, [1, 128])
            iota_col = load(iota_col_d, [128, 512])
            iota_part = load(iota_part_d, [128, 1])
            ident_bf = load(ident_bf_d, [128, 128], bf16)
            hsl_t = pp.tile([128, NWIN, 128], f32, tag="h_slice")
            nc.sync.dma_start(hsl_t[:], h_slice.ap().rearrange("w p d -> p w d"))

            # ---- B table: h @ we1_bot -> bf16 rows in HBM ----
            bview = B_hbm.ap().rearrange("(g t p) d -> g p t d", g=10, t=8, p=128)
            for g in range(10):
                stage = wp.tile([128, 8, 128], bf16, tag="bstage")
                for ts in range(8):
                    t = g * 8 + ts
                    bp = psp.tile([128, 128], f32, tag="e1")
                    nc.tensor.matmul(bp[:], hT_t[:, t * 128:(t + 1) * 128], w1b[:],
                                     start=True, stop=True)
                    nc.scalar.activation(stage[:, ts, :], bp[:], AF.Copy)
                nc.sync.dma_start(bview[g], stage[:])

            # ---- A table: h @ we1_top + be1, double-bf16, SBUF resident ----
            a_hi = pp.tile([128, NWIN, 128], bf16, tag="a_hi")
            a_lo = pp.tile([128, NWIN, 128], bf16, tag="a_lo")
            for w in range(NWIN):
                ap_ = psp.tile([128, 128], f32, tag="e1")
                nc.tensor.matmul(ap_[:], onesr[:], be1r[:], start=True, stop=False)
                nc.tensor.matmul(ap_[:], hTs_t[:, w * 128:(w + 1) * 128], w1t[:],
                                 start=False, stop=True)
                nc.scalar.activation(a_hi[:, w, :], ap_[:], AF.Copy)
                nc.vector.tensor_tensor(
                    a_lo[:, w, :], ap_[:], a_hi[:, w, :], ALU.subtract)

            # ---- edge phase ----
            agg_sb = pp.tile([128, NWIN, 128], f32, tag="aggsb")
            m = 0
            c = 0
            for w in range(NWIN):
                agg_ps = psp.tile([128, 128], f32, tag="agg")
                nmw = nm_w[w]
                gt = None
                for mw in range(nmw):
                    if mw % 2 == 0:
                        gt = gp.tile([128, 1, CALL], bf16, tag="g")
                        nc.gpsimd.dma_gather(
                            gt[:], B_hbm.ap(), colidx_t[:, c * 64:(c + 1) * 64],
                            num_idxs=CALL, num_idxs_reg=CALL, elem_size=128,
                            transpose=True, single_packet=False,
                        )
                        c += 1
                    plane = gt[:, 0, (mw % 2) * 512:(mw % 2) * 512 + 512]

                    rb = wp.tile([128, 512], f32, tag="rb")
                    nc.sync.dma_start(
                        rb[:], rowloc_r_d.ap()[m:m + 1, :].broadcast_to((128, 512)))
                    st = wp.tile([128, 512], bf16, tag="st")
                    nc.vector.tensor_scalar(
                        st[:], rb[:], iota_part[:, 0:1], None, ALU.is_equal)

                    e1p = psp.tile([128, 512], f32, tag="e1")
                    nc.tensor.matmul(e1p[:], a_hi[:, w, :], st[:],
                                     start=True, stop=False, skip_group_check=True)
                    nc.tensor.matmul(e1p[:], a_lo[:, w, :], st[:],
                                     start=False, stop=False, skip_group_check=True)
                    nc.tensor.matmul(e1p[:], ident_bf[:], plane,
                                     start=False, stop=True, skip_group_check=True)
                    e1s = wp.tile([128, 512], f32, tag="e1s")
                    nc.scalar.activation(e1s[:], e1p[:], AF.Silu)

                    s4 = wp.tile([128, 512], f32, tag="s4")
                    for t in range(4):
                        nc.vector.tensor_scalar(
                            s4[:, t * 128:(t + 1) * 128],
                            iota_col[:, t * 128:(t + 1) * 128],
                            rowloc_c[:, 4 * m + t:4 * m + t + 1],
                            None, ALU.is_equal)

                    e2p = psp.tile([128, 512], f32, tag="e2")
                    nc.tensor.matmul(e2p[:], onesr[:], be2r[:],
                                     start=True, stop=False, skip_group_check=True)
                    for t in range(4):
                        nc.tensor.matmul(
                            e2p[:, t * 128:(t + 1) * 128],
                            e1s[:, t * 128:(t + 1) * 128], w2[:],
                            start=False, stop=True, skip_group_check=True)
                    e2s = wp.tile([128, 512], f32, tag="e2s")
                    nc.scalar.activation(e2s[:], e2p[:], AF.Silu)

                    for t in range(4):
                        nc.tensor.matmul(
                            agg_ps[:],
                            e2s[:, t * 128:(t + 1) * 128],
                            s4[:, t * 128:(t + 1) * 128],
                            start=(mw == 0 and t == 0),
                            stop=(mw == nmw - 1 and t == 3),
                            skip_group_check=True)
                    m += 1
                nc.scalar.activation(agg_sb[:, w, :], agg_ps[:], AF.Copy,
                                     scale=1.0 / NORM)

            # ---- node phase ----
            for w in range(NWIN):
                hp = psp.tile([128, 128], f32, tag="e1")
                nc.tensor.matmul(hp[:], wn1l[:], hTs_t[:, w * 128:(w + 1) * 128],
                                 start=True, stop=False)
                nc.tensor.matmul(hp[:], wn1h[:], agg_sb[:, w, :],
                                 start=False, stop=True)
                hs = wp.tile([128, 128], f32, tag="hs")
                nc.scalar.activation(hs[:], hp[:], AF.Silu, bias=bn1c[:, 0:1])
                op = psp.tile([128, 128], f32, tag="e2")
                nc.tensor.matmul(op[:], onesr[:], bn2r[:], start=True, stop=False)
                nc.tensor.matmul(op[:], hs[:], wn2t[:], start=False, stop=True)
                ot = wp.tile([128, 128], f32, tag="ot")
                nc.vector.tensor_tensor(ot[:], op[:], hsl_t[:, w, :], ALU.add)
                nc.sync.dma_start(out_d.ap()[w], ot[:])

    nc.compile()
    return nc


def _prep_inputs(h, edge_index, we1, be1, we2, be2, wn1, bn1, wn2, bn2):
    """Host-side shard/sort/pad. Returns (cw_per_window, per-core in_maps)."""
    h = np.asarray(h, np.float32)
    row = np.asarray(edge_index[0], np.int64).astype(np.int32)
    col = np.asarray(edge_index[1], np.int64).astype(np.int32)

    # per (core, window) edge lists
    core = row // NPC
    rl_g = row - core * NPC
    win = rl_g // 128
    rl = (rl_g % 128).astype(np.float32)

    counts = np.zeros((NCORES, NWIN), np.int64)
    per = [[None] * NWIN for _ in range(NCORES)]
    for cid in range(NCORES):
        msk = core == cid
        w_c, rl_c, col_c = win[msk], rl[msk], col[msk]
        for w in range(NWIN):
            wm = w_c == w
            per[cid][w] = (col_c[wm], rl_c[wm])
            counts[cid, w] = wm.sum()
    cw_per_window = tuple(int(-(-counts[:, w].max() // CALL)) for w in range(NWIN))

    nm_w = [2 * c for c in cw_per_window]
    NM = sum(nm_w)
    NCALLS = sum(cw_per_window)
    RBLK = (NM + 127) // 128

    hT_pad = np.zeros((128, 10240), np.float32)
    hT_pad[:, :N_NODES] = h.T
    iota_col = np.tile(np.arange(128, dtype=np.float32), 4)[None, :].repeat(128, 0)
    iota_part = np.arange(128, dtype=np.float32)[:, None].copy()
    ident_bf = np.eye(128, dtype=np.float32).astype(BF16)
    shared = {
        "hT": hT_pad,
        "we1_top": np.asarray(we1[:128], np.float32),
        "we1_bot": np.asarray(we1[128:], np.float32),
        "be1_row": np.asarray(be1, np.float32)[None, :],
        "we2": np.asarray(we2, np.float32),
        "be2rep4": np.tile(np.asarray(be2, np.float32), 4)[None, :],
        "wn1_lo": np.asarray(wn1[:128], np.float32),
        "wn1_hi": np.asarray(wn1[128:], np.float32),
        "bn1_col": np.asarray(bn1, np.float32)[:, None].copy(),
        "wn2": np.asarray(wn2, np.float32),
        "bn2_row": np.asarray(bn2, np.float32)[None, :],
        "ones_row": np.ones((1, 128), np.float32),
        "iota_col": iota_col.copy(),
        "iota_part": iota_part,
        "ident_bf": ident_bf,
    }

    in_maps = []
    for cid in range(NCORES):
        colidx = np.zeros((128, 64 * NCALLS), np.int16)
        rowloc_c = np.zeros((128, 4 * NM), np.float32)
        rowloc_r = np.zeros((NM, 512), np.float32)
        ci = 0
        mi = 0
        for w in range(NWIN):
            ccol, crl = per[cid][w]
            n_slots = cw_per_window[w] * CALL
            col_pad = np.zeros(n_slots, np.int32)
            rl_pad = np.full(n_slots, PAD_ROWLOCAL, np.float32)
            col_pad[:len(ccol)] = ccol
            rl_pad[:len(crl)] = crl
            for cc in range(cw_per_window[w]):
                colidx[:, ci * 64:ci * 64 + 64] = _wrap_idx16(
                    col_pad[cc * CALL:(cc + 1) * CALL])
                ci += 1
            for mm in range(2 * cw_per_window[w]):
                seg = rl_pad[mm * MACRO:(mm + 1) * MACRO]
                rowloc_c[:, 4 * mi:4 * mi + 4] = seg.reshape(4, 128).T
                rowloc_r[mi] = seg
                mi += 1
        base = cid * NPC
        hT_slice = hT_pad[:, base:base + NWIN * 128].copy()
        h_slice = np.zeros((NWIN, 128, 128), np.float32)
        hi = min(N_NODES, base + NWIN * 128)
        h_slice.reshape(NWIN * 128, 128)[:hi - base] = h[base:hi]
        in_maps.append({**shared, "hT_slice": hT_slice, "h_slice": h_slice,
                        "colidx": colidx, "rowloc_c": rowloc_c,
                        "rowloc_r": rowloc_r})
    return cw_per_window, in_maps


def kernel(**inputs):
    from concourse.bass_utils import run_bass_kernel_spmd

    cw, in_maps = _prep_inputs(**inputs)
    if cw not in _prog_cache:
        _prog_cache[cw] = _build_program(cw)
    nc = _prog_cache[cw]
    res = run_bass_kernel_spmd(nc, in_maps, list(range(NCORES)))
    outs = []
    for cid in range(NCORES):
        o = res.results[cid]["out"].reshape(NWIN * 128, 128)
        outs.append(o[:NPC])
    return np.concatenate(outs, axis=0)[:N_NODES].astype(np.float32)
